# revision 34
# baseline (speedup 1.0000x reference)
"""BitFeedForward (BitNet b1.58 MLP) Trainium2 kernel — 8-core data-parallel.

Reference computation (per token row t of x [B*S, D]):
  xq  = round(x * sx) / sx            sx = 127/clip(absmax_row, EPS)
  wq1 = clip(round(w1/u1), -1, 1)*u1  u1 = clip(mean|w1|, EPS)   (per tensor)
  h   = xq @ wq1.T + b1
  g   = gelu(h)  (erf)
  hn  = (g - mu)/sqrt(var + EPS) * gamma + beta     (ln over F)
  hq  = round(hn * sh) / sh           sh = 127/clip(absmax_row(hn), EPS)
  y   = hq @ wq2.T + b2

Numeric facts: quantized activations are integers in [-127,127] (exact
in bf16) and quantized weights are ternary {-1,0,1} (exact in fp8e4);
PSUM accumulates fp32, so both matmuls run at the full bf16 PE rate
with exact integer arithmetic; scales fold into evict/elementwise ops.
Rounding uses the +-1.5*2^23 magic-constant trick (round-half-even,
matching jnp.round). Host prep ternarizes + transposes the weights
(deployment constants in BitNet); everything per-token runs on device.

V2 design ("F-major mm1"): mm1 is computed TRANSPOSED — stationary =
w1 chunks (fp8), moving = xqT — so its output lands as h^T [F, tok] in
exactly the layout mm2 needs for its stationary operand. This kills
the baseline's g/hq DRAM round-trips and the mm1->mm2 DMA-transpose
stall entirely. Per-token LN stats (sum/sumsq/max over F; the gelu
min branch is always dominated, see emit_C) are kept as running
per-F-lane accumulators updated per 128-F chunk (DVE + ACT, with only
the sumsq-accumulate on GPSIMD — real-HW GPSIMD tensor ops measured
~2x the cost model, so bulk elementwise stays on DVE), then collapsed
across partitions at sweep end with gpsimd partition_all_reduce (6
ops total). mm1 runs in two token-tile sweeps (tokens 0:512,
512:1024) so sweep-0's stats collapse, coefficient math, and in-place
quantization of gq overlap sweep-1's matmuls. mm2 (n2-major, k inner,
8 PSUM banks) streams w2 straight from DRAM; its first n2 pass is
split into two 4-bank m-halves so the tt=1 quant races ahead of the
PE's consumption without ever stalling the (strictly in-order) PE
queue. Weight DMA: w1 streamed twice (once per sweep), w2 ~1.25x.
Everything per-token stays in SBUF end to end.

Sharding: data-parallel over the 8192 token rows -> 1024 tokens/core,
no collectives.
"""

import os
import numpy as np
import ml_dtypes

B_DIM, S_DIM, D_DIM, F_DIM = 4, 2048, 2048, 8192
N_CORES = 8
TOK = B_DIM * S_DIM           # 8192 total tokens
T = TOK // N_CORES            # 1024 tokens per core
P = 128
MB = T // P                   # 8 token blocks per core
KD = D_DIM // P               # 16 contraction chunks for mm1
CF = F_DIM // P               # 64 F chunks (mm1 out partition / mm2 contraction)
ND2 = D_DIM // 512            # 4 D tiles (mm2 output)
NT = 2                        # token tiles of 512 (mm1 moving dim)
TT = T // NT                  # 512
EPS = 1e-5
MAGIC = 12582912.0            # 1.5 * 2**23: (x + MAGIC) - MAGIC == rint(x)

_CACHE: dict = {}


def _build_program(use_gelu: bool = True, reps: int = 1):
    import concourse.bass as bass
    import concourse.bass_isa as bass_isa
    import concourse.mybir as mybir
    import concourse.tile as tile
    from concourse import bacc
    from concourse.bass import ts, ds

    f32 = mybir.dt.float32
    bf16 = mybir.dt.bfloat16
    fp8 = mybir.dt.float8e4
    AF = mybir.ActivationFunctionType
    ALU = mybir.AluOpType
    AX = mybir.AxisListType

    nc = bacc.Bacc("TRN2", target_bir_lowering=False, debug=False,
                   num_devices=N_CORES)

    x_d = nc.dram_tensor("x", [T, D_DIM], f32, kind="ExternalInput")
    w1t_d = nc.dram_tensor("w1t", [D_DIM, F_DIM], fp8, kind="ExternalInput")
    w2t_d = nc.dram_tensor("w2t", [F_DIM, D_DIM], fp8, kind="ExternalInput")
    b1_d = nc.dram_tensor("b1", [F_DIM], f32, kind="ExternalInput")
    b2_d = nc.dram_tensor("b2", [D_DIM], bf16, kind="ExternalInput")
    wsc_d = nc.dram_tensor("wsc", [2], f32, kind="ExternalInput")
    y_d = nc.dram_tensor("y", [T, D_DIM], f32, kind="ExternalOutput")

    def bcast_ap(t):
        ap = t if isinstance(t, bass.AP) else t.ap()
        return bass.AP(tensor=ap.tensor, offset=ap.offset,
                       ap=[[0, P]] + list(ap.ap))

    x_ap = x_d.ap()
    y_ap = y_d.ap()
    w1_v = w1t_d.ap().rearrange("(o p) f -> p o f", p=P)   # [128,16,F]
    w2_v = w2t_d.ap().rearrange("(o p) d -> p o d", p=P)   # [128,64,D]

    with tile.TileContext(nc) as tc:
        with (
            tc.tile_pool(name="const", bufs=1) as const,
            tc.tile_pool(name="dram", bufs=1, space="DRAM") as dram,
        ):
            wsc_t = const.tile([P, 2], f32)
            nc.sync.dma_start(out=wsc_t[:], in_=bcast_ap(wsc_d))
            eps_t = const.tile([P, 1], f32)
            nc.vector.memset(eps_t[:], EPS)
            b1t = const.tile([P, CF], f32)       # b1 as [F-lane, chunk]
            nc.sync.dma_start(out=b1t[:],
                              in_=b1_d.ap().rearrange("(c p) -> p c", p=P))
            xq_dram = [dram.tile([P, D_DIM], bf16, name=f"xqd{m}")
                       for m in range(MB)]
            cdram = dram.tile([1, T], f32, name="vs2row")
            vdram = dram.tile([1, T], bf16, name="vs1bf")

            for rep in range(reps):
                from contextlib import ExitStack

                stk = ExitStack()
                keep = stk.enter_context(
                    tc.tile_pool(name=f"keep_{rep}", bufs=1, side="right"))
                xqT = keep.tile([P, KD, T], bf16, name=f"xqT_{rep}")
                gq = keep.tile([P, CF, T], bf16, name=f"gq_{rep}")
                rsum = keep.tile([P, T], f32, name=f"rsum_{rep}")
                rsq = keep.tile([P, T], f32, name=f"rsq_{rep}")
                rmx = keep.tile([P, T], bf16, name=f"rmx_{rep}")
                vs1rep = keep.tile([P, T], bf16, name=f"vs1rep_{rep}")
                vs1tok = keep.tile([P, MB], f32, name=f"vs1tok_{rep}")
                vs2tok = keep.tile([P, MB], f32, name=f"vs2tok_{rep}")
                acoef = keep.tile([P, MB], f32, name=f"acoef_{rep}")
                btok = keep.tile([P, MB], f32, name=f"btok_{rep}")

                nc.vector.memset(rsum[:], 0.0)
                nc.vector.memset(rsq[:], 0.0)
                nc.vector.memset(rmx[:], -3.0e38)

                # pools for mm1 sweeps + quant (opened before A so the
                # A-phase pools can close first: LIFO pool stack)
                stq = ExitStack()
                keep2 = stq.enter_context(
                    tc.tile_pool(name=f"keep2_{rep}", bufs=1, side="right"))
                arep = keep2.tile([P, T], f32, name=f"arep_{rep}")
                brep = keep2.tile([P, T], f32, name=f"brep_{rep}")
                pqt = stq.enter_context(
                    tc.tile_pool(name=f"pqt_{rep}", bufs=2))
                stb = ExitStack()
                pw1 = stb.enter_context(
                    tc.tile_pool(name=f"pw1_{rep}", bufs=2))
                pev = stb.enter_context(
                    tc.tile_pool(name=f"pev_{rep}", bufs=2))
                pc = stb.enter_context(
                    tc.tile_pool(name=f"pc_{rep}", bufs=1))
                psum1 = stb.enter_context(
                    tc.tile_pool(name=f"psum1_{rep}", bufs=4, space="PSUM"))

                # ---------------- A phase: x -> xq -> xqT ----------------
                sta = ExitStack()
                pa = sta.enter_context(
                    tc.tile_pool(name=f"pa_{rep}", bufs=2))
                pq = sta.enter_context(
                    tc.tile_pool(name=f"pq_{rep}", bufs=1))
                ps = sta.enter_context(
                    tc.tile_pool(name=f"psm_{rep}", bufs=2))

                HD = D_DIM // 2

                def emit_A(m):
                    xth = []
                    engs = (nc.vector, nc.vector)
                    qs = (nc.scalar, nc.sync)
                    am2 = ps.tile([P, 2], f32, tag="am2", name="am2")
                    for h in range(2):
                        xt = pa.tile([P, HD], f32, tag="xt", name="xt")
                        qs[h].dma_start(
                            xt[:], x_ap[m * P:(m + 1) * P, ts(h, HD)])
                        nc.vector.tensor_reduce(am2[:, h:h + 1], xt[:],
                                                axis=AX.X, op=ALU.max,
                                                apply_absolute_value=True)
                        xth.append(xt)
                    am = ps.tile([P, 1], f32, tag="am", name="am")
                    nc.vector.tensor_max(am[:], am2[:, 0:1], am2[:, 1:2])
                    nc.vector.tensor_scalar_max(am[:], am[:], EPS)
                    vx = ps.tile([P, 1], f32, tag="vx", name="vx")
                    nc.vector.tensor_scalar_mul(vx[:], am[:], 1.0 / 127.0)
                    nc.vector.tensor_mul(vs1tok[:, m:m + 1], vx[:],
                                         wsc_t[:, 0:1])
                    sx = ps.tile([P, 1], f32, tag="sx", name="sx")
                    nc.vector.reciprocal(sx[:], vx[:])
                    for h in range(2):
                        engs[h].tensor_scalar(xth[h][:], xth[h][:], sx[:],
                                              MAGIC, ALU.mult, ALU.add)
                        xqh = pq.tile([P, HD], bf16, tag="xqh", name="xqh")
                        engs[h].tensor_scalar(xqh[:], xth[h][:], MAGIC,
                                              None, ALU.subtract)
                        qs[1 - h].dma_start(xq_dram[m][:, ts(h, HD)],
                                            xqh[:])
                    nc.sync.dma_start_transpose(
                        xqT[:, :, ts(m, P)], xq_dram[m][:])

                def bounce_vs1(tt):
                    vb = ps.tile([P, NT * 2], bf16, tag="vb", name="vb")
                    sl = slice(tt * (MB // NT), (tt + 1) * (MB // NT))
                    nc.vector.tensor_copy(vb[:, 0:MB // NT], vs1tok[:, sl])
                    nc.sync.dma_start(
                        vdram[0, ts(tt, TT)].rearrange("(j p) -> p j", p=P),
                        vb[:, 0:MB // NT])
                    nc.scalar.dma_start(vs1rep[:, ts(tt, TT)],
                                        bcast_ap(vdram[0, ts(tt, TT)]))

                for m in range(MB):
                    emit_A(m)
                    if m == MB // NT - 1:
                        bounce_vs1(0)
                bounce_vs1(1)

                # ---------------- mm1 sweeps + stats + quant -------------
                def emit_mm1_chunk(tt, c, split=False):
                    w1c = pw1.tile([P, KD, P], fp8, tag="w1c", name="w1c")
                    nc.scalar.dma_start(w1c[:], w1_v[:, :, ts(c, P)])
                    pt = psum1.tile([P, TT], f32, tag="ps1", name="ps1")
                    if split:
                        # per-block N=128 moving slices: lets the PE start
                        # as soon as block b's xqT transpose lands instead
                        # of waiting for the full 512-token tile
                        for b in range(TT // P):
                            tb = tt * (TT // P) + b
                            for k in range(KD):
                                nc.tensor.matmul(pt[:, ts(b, P)],
                                                 w1c[:, k, :],
                                                 xqT[:, k, ts(tb, P)],
                                                 start=(k == 0),
                                                 stop=(k == KD - 1))
                    else:
                        for k in range(KD):
                            nc.tensor.matmul(pt[:], w1c[:, k, :],
                                             xqT[:, k, ts(tt, TT)],
                                             start=(k == 0),
                                             stop=(k == KD - 1))
                    tsl = ts(tt, TT)
                    tmp = pev.tile([P, TT], f32, tag="tmp", name="tmp")
                    nc.vector.tensor_mul(tmp[:], pt[:], vs1rep[:, tsl])
                    gsl = gq[:, c, tsl]
                    nc.scalar.activation(gsl, tmp[:],
                                         AF.Gelu if use_gelu else AF.Identity,
                                         bias=b1t[:, c:c + 1])
                    sq = pev.tile([P, TT], bf16, tag="sq", name="sq")
                    nc.scalar.activation(sq[:], gsl, AF.Square)
                    nc.vector.tensor_add(rsum[:, tsl], rsum[:, tsl], gsl)
                    nc.gpsimd.tensor_add(rsq[:, tsl], rsq[:, tsl], sq[:])
                    nc.vector.tensor_max(rmx[:, tsl], rmx[:, tsl], gsl)

                def emit_C(tt):
                    """Stat collapse (in-place gpsimd partition
                    all-reduce: every partition ends up holding the
                    per-token total) + LN/quant coefficient math directly
                    on broadcast-form [P, 512] slices. arep/brep slices
                    double as scratch before receiving their final
                    values; vs2 (needed in token layout by mm2's evict)
                    bounces through DRAM off the critical path."""
                    tsl = ts(tt, TT)
                    sl = slice(tt * (MB // NT), (tt + 1) * (MB // NT))
                    RO = bass_isa.ReduceOp
                    mu = rsum[:, tsl]
                    va = rsq[:, tsl]
                    asl = arep[:, tsl]
                    bsl = brep[:, tsl]
                    nc.gpsimd.partition_all_reduce(mu, mu, P, RO.add)
                    nc.gpsimd.partition_all_reduce(va, va, P, RO.add)
                    nc.gpsimd.partition_all_reduce(rmx[:, tsl],
                                                   rmx[:, tsl], P, RO.max)
                    cs = pc.tile([P, TT], f32, tag="cs", name="cs")
                    nc.vector.tensor_scalar_mul(mu, mu, 1.0 / F_DIM)
                    nc.vector.tensor_scalar_mul(va, va, 1.0 / F_DIM)
                    nc.vector.tensor_mul(cs[:], mu, mu)
                    nc.vector.tensor_sub(va, va, cs[:])
                    # cs := rstd = 1/sqrt(var + eps)
                    nc.scalar.activation(cs[:], va, AF.Sqrt, bias=eps_t[:])
                    nc.vector.reciprocal(cs[:], cs[:])
                    # asl (scratch) := gmax - mu ; bsl := amh = rstd*(gmax-mu)
                    nc.vector.tensor_sub(asl, rmx[:, tsl], mu)
                    nc.vector.tensor_mul(bsl, asl, cs[:])
                    nc.vector.tensor_scalar_max(bsl, bsl, EPS)
                    # va := sh = 127/amh
                    nc.vector.reciprocal(va, bsl)
                    nc.vector.tensor_scalar_mul(va, va, 127.0)
                    # vs2 = (amh/127)*u2 -> DRAM -> token layout (later)
                    nc.vector.tensor_scalar(bsl, bsl, wsc_t[:, 1:2],
                                            1.0 / 127.0, ALU.mult, ALU.mult)
                    nc.sync.dma_start(cdram[0, tsl], bsl[0:1, :])
                    nc.sync.dma_start(
                        vs2tok[:, sl],
                        cdram[0, tsl].rearrange("(j p) -> p j", p=P))
                    # A = rstd*sh ; B = -mu*A
                    nc.vector.tensor_mul(asl, cs[:], va)
                    nc.vector.scalar_tensor_tensor(bsl, mu, -1.0, asl,
                                                   ALU.mult, ALU.mult)

                def emit_quant(tt, c, eng):
                    tsl = ts(tt, TT)
                    gsl = gq[:, c, tsl]
                    q1 = pqt.tile([P, TT], f32, tag="q1", name="q1")
                    eng.tensor_mul(q1[:], gsl, arep[:, tsl])
                    eng.tensor_add(q1[:], q1[:], brep[:, tsl])
                    eng.tensor_scalar(gsl, q1[:], MAGIC, MAGIC,
                                      ALU.add, ALU.subtract)

                # sweep 0 (tokens 0:512); first chunks split per block
                for c in range(CF):
                    emit_mm1_chunk(0, c, split=(c < 6))
                sta.close()
                emit_C(0)
                # sweep 1; tt=0 quant interleaves (scheduler overlaps them)
                for c in range(CF):
                    emit_mm1_chunk(1, c)
                    emit_quant(0, c, nc.vector)
                emit_C(1)
                stb.close()

                # ---------------- mm2 ----------------
                # (pools open BEFORE the tt=1 quant emission: a pool that
                # opens later takes a conservative fence on everything
                # already emitted, which would chain the first w2 loads
                # behind the whole quant)
                stc = ExitStack()
                pw2 = stc.enter_context(
                    tc.tile_pool(name=f"pw2_{rep}", bufs=4))
                py = stc.enter_context(
                    tc.tile_pool(name=f"py_{rep}", bufs=3))
                pb2 = stc.enter_context(
                    tc.tile_pool(name=f"pb2_{rep}", bufs=1))
                b2r = pb2.tile([P, D_DIM], bf16, name=f"b2r_{rep}")
                nc.sync.dma_start(out=b2r[:], in_=bcast_ap(b2_d))
                psum2 = stc.enter_context(
                    tc.tile_pool(name=f"psum2_{rep}", bufs=1, space="PSUM"))

                # tt=1 quant: split across DVE and GPSIMD to outrun mm2
                for c in range(CF):
                    emit_quant(1, c, nc.vector)

                def emit_mm2_pass(n2, ms):
                    pts = {m: psum2.tile([P, 512], f32, tag=f"e{m}",
                                         name=f"e{n2}_{m}")
                           for m in ms}
                    for c in range(CF):
                        w2a = pw2.tile([P, 512], fp8, tag="w2a", name="w2a")
                        nc.scalar.dma_start(w2a[:], w2_v[:, c, ts(n2, 512)])
                        for m in ms:
                            nc.tensor.matmul(pts[m][:],
                                             gq[:, c, ts(m, P)], w2a[:],
                                             start=(c == 0),
                                             stop=(c == CF - 1))
                    for m in ms:
                        yt = py.tile([P, 512], f32, tag="yt", name="yt")
                        nc.vector.scalar_tensor_tensor(
                            yt[:], pts[m][:], vs2tok[:, m:m + 1],
                            b2r[:, ts(n2, 512)], ALU.mult, ALU.add)
                        nc.sync.dma_start(
                            y_ap[m * P:(m + 1) * P, ts(n2, 512)], yt[:])

                # tt0-only passes first: ~110us of PE work before any
                # tt=1-quant dependency reaches the in-order PE queue;
                # last pass split so its first half's evicts/stores
                # overlap the second half's MMs
                lo = list(range(MB // 2))
                hi = list(range(MB // 2, MB))
                emit_mm2_pass(0, lo)
                emit_mm2_pass(1, lo)
                emit_mm2_pass(0, hi)
                emit_mm2_pass(1, hi)
                emit_mm2_pass(2, lo + hi)
                emit_mm2_pass(ND2 - 1, lo)
                emit_mm2_pass(ND2 - 1, hi)
                stc.close()
                stq.close()
                stk.close()

    nc.compile()
    return nc


def _get_runner(reps: int = 1):
    """Build (once) a jitted 8-core shard_map executor for the program."""
    key = ("runner", reps)
    if key in _CACHE:
        return _CACHE[key]

    import jax
    import numpy as np
    import concourse.mybir as mybir
    from concourse import bass2jax
    from jax.experimental.shard_map import shard_map
    from jax.sharding import Mesh, PartitionSpec

    nc = _build_program(reps=reps)
    bass2jax.install_neuronx_cc_hook()

    partition_name = (nc.partition_id_tensor.name
                      if nc.partition_id_tensor else None)
    in_names: list[str] = []
    out_names: list[str] = []
    out_avals = []
    zero_outs: list[np.ndarray] = []
    for alloc in nc.m.functions[0].allocations:
        if not isinstance(alloc, mybir.MemoryLocationSet):
            continue
        name = alloc.memorylocations[0].name
        if alloc.kind == "ExternalInput":
            if name != partition_name:
                in_names.append(name)
        elif alloc.kind == "ExternalOutput":
            shape = tuple(alloc.tensor_shape)
            dtype = mybir.dt.np(alloc.dtype)
            out_names.append(name)
            out_avals.append(jax.core.ShapedArray(shape, dtype))
            zero_outs.append(np.zeros(shape, dtype))
    n_params = len(in_names)
    n_outs = len(out_avals)
    in_names = in_names + out_names
    if partition_name is not None:
        in_names.append(partition_name)

    def _body(*args):
        operands = list(args)
        if partition_name is not None:
            operands.append(bass2jax.partition_id_tensor())
        outs = bass2jax._bass_exec_p.bind(
            *operands,
            out_avals=tuple(out_avals),
            in_names=tuple(in_names),
            out_names=tuple(out_names),
            lowering_input_output_aliases=(),
            sim_require_finite=True,
            sim_require_nnan=True,
            nc=nc,
        )
        return tuple(outs)

    devices = jax.devices()[:N_CORES]
    assert len(devices) == N_CORES, f"need {N_CORES} devices"
    mesh = Mesh(np.asarray(devices), ("core",))
    in_specs = (PartitionSpec("core"),) * (n_params + n_outs)
    out_specs = (PartitionSpec("core"),) * n_outs
    sharded = jax.jit(shard_map(_body, mesh=mesh, in_specs=in_specs,
                                out_specs=out_specs, check_rep=False),
                      keep_unused=True)

    runner = {
        "nc": nc, "sharded": sharded, "mesh": mesh,
        "in_names": in_names[:n_params], "out_names": out_names,
        "out_avals": out_avals, "zero_outs": zero_outs,
    }
    _CACHE[key] = runner
    return runner


def _host_prep(x, w1, b1, gamma, beta, w2, b2):
    """Ternarize + transpose weights on host; build per-core input list."""
    f32 = np.float32
    u1 = f32(np.clip(np.mean(np.abs(w1), dtype=f32), EPS, None))
    u2 = f32(np.clip(np.mean(np.abs(w2), dtype=f32), EPS, None))
    s1 = f32(1.0) / u1
    s2 = f32(1.0) / u2
    t1 = np.clip(np.round(w1.astype(f32) * s1), -1.0, 1.0)
    t2 = np.clip(np.round(w2.astype(f32) * s2), -1.0, 1.0)
    w1t = np.ascontiguousarray(t1.T).astype(ml_dtypes.float8_e4m3fn)  # [D,F]
    w2t = np.ascontiguousarray(t2.T).astype(ml_dtypes.float8_e4m3fn)  # [F,D]
    wsc = np.array([u1, u2], dtype=f32)
    ident = np.eye(P, dtype=f32)
    xf = np.ascontiguousarray(x.reshape(TOK, D_DIM).astype(f32))
    shards = [xf[c * T:(c + 1) * T] for c in range(N_CORES)]
    b1f = b1.astype(f32)
    b2f = b2.astype(ml_dtypes.bfloat16)
    return [{"x": shards[c], "w1t": w1t, "w2t": w2t,
             "b1": b1f, "b2": b2f, "wsc": wsc, "ident": ident}
            for c in range(N_CORES)]


def _concat_inputs(runner, in_maps):
    return [np.concatenate([np.asarray(in_maps[c][name])
                            for c in range(N_CORES)], axis=0)
            for name in runner["in_names"]]


def _run_once(runner, concat_in):
    import numpy as np
    zeros = [np.zeros((N_CORES * z.shape[0], *z.shape[1:]), z.dtype)
             for z in runner["zero_outs"]]
    out_arrs = runner["sharded"](*concat_in, *zeros)
    (yname,) = runner["out_names"]
    (yaval,) = runner["out_avals"]
    y_all = np.asarray(out_arrs[0]).reshape(N_CORES, *yaval.shape)
    return y_all


def _fallback_numpy(x, w1, b1, gamma, beta, w2, b2):
    """Reference-faithful host fallback (only for inputs the compiled
    program isn't specialized for, e.g. non-trivial gamma/beta)."""
    import jax
    with jax.default_device(jax.devices("cpu")[0]):
        import jax.numpy as jnp

        def aq(v):
            sc = 127.0 / jnp.clip(jnp.max(jnp.abs(v), axis=-1,
                                          keepdims=True), EPS, None)
            return jnp.clip(jnp.round(v * sc), -128.0, 127.0) / sc

        def wq(w):
            sc = 1.0 / jnp.clip(jnp.mean(jnp.abs(w)), EPS, None)
            return jnp.clip(jnp.round(w * sc), -1.0, 1.0) / sc

        h = jnp.einsum('bsd,fd->bsf', aq(jnp.asarray(x)), wq(jnp.asarray(w1))) + b1
        h = jax.nn.gelu(h, approximate=False)
        mu = jnp.mean(h, axis=-1, keepdims=True)
        var = jnp.var(h, axis=-1, keepdims=True)
        h = (h - mu) * jax.lax.rsqrt(var + EPS) * gamma + beta
        out = jnp.einsum('bsf,df->bsd', aq(h), wq(jnp.asarray(w2))) + b2
        return np.asarray(out, dtype=np.float32)


def kernel(x, w1, b1, gamma, beta, w2, b2):
    x = np.asarray(x)
    w1 = np.asarray(w1)
    b1 = np.asarray(b1)
    gamma = np.asarray(gamma)
    beta = np.asarray(beta)
    w2 = np.asarray(w2)
    b2 = np.asarray(b2)

    shapes_ok = (x.shape == (B_DIM, S_DIM, D_DIM)
                 and w1.shape == (F_DIM, D_DIM)
                 and w2.shape == (D_DIM, F_DIM))
    ln_trivial = bool(np.all(gamma == 1.0) and np.all(beta == 0.0))
    if not (shapes_ok and ln_trivial):
        return _fallback_numpy(x, w1, b1, gamma, beta, w2, b2)

    runner = _get_runner()
    in_maps = _host_prep(x, w1, b1, gamma, beta, w2, b2)
    y_all = _run_once(runner, _concat_inputs(runner, in_maps))
    return y_all.reshape(TOK, D_DIM).reshape(B_DIM, S_DIM, D_DIM)


def bench_delta(inputs, reps=4, trials=6, iters=(6, 20)):
    """Measure per-pipeline device time: build a NEFF with the pipeline
    repeated `reps` times (intra-NEFF work is strictly serial on-device),
    amortize dispatch with pipelined async calls, and take
    marginal-wall-time/reps. Min over trials rejects contention noise on
    the shared device; marginal/reps includes inter-call gaps, so it is a
    conservative (over-) estimate. Returns (y_full, per_pipeline_ns)."""
    import time
    import jax
    from jax.sharding import NamedSharding, PartitionSpec

    in_maps = _host_prep(**inputs)
    runner = _get_runner(reps=reps)
    concat_in = _concat_inputs(runner, in_maps)
    sharding = NamedSharding(runner["mesh"], PartitionSpec("core"))
    dev_in = [jax.device_put(a, sharding) for a in concat_in]
    zeros = [np.zeros((N_CORES * z.shape[0], *z.shape[1:]), z.dtype)
             for z in runner["zero_outs"]]
    dev_zeros = [jax.device_put(z, sharding) for z in zeros]
    f = runner["sharded"]
    o = f(*dev_in, *dev_zeros)
    jax.block_until_ready(o)
    (yaval,) = runner["out_avals"]
    y_all = np.asarray(o[0]).reshape(N_CORES, *yaval.shape)
    y = y_all.reshape(TOK, D_DIM).reshape(B_DIM, S_DIM, D_DIM)

    samples = []
    for _ in range(trials):
        tsd = {}
        for it in iters:
            t0 = time.perf_counter()
            ks = [f(*dev_in, *dev_zeros) for _ in range(it)]
            jax.block_until_ready(ks[-1])
            tsd[it] = time.perf_counter() - t0
        m = (tsd[iters[1]] - tsd[iters[0]]) / (iters[1] - iters[0])
        samples.append(m / reps * 1e9)
    samples.sort()
    print(f"bench_delta samples (ns): {[f'{s:.0f}' for s in samples]}")
    # median: robust to both contention outliers (high) and cross-call
    # on-device overlap artifacts (impossibly low, below the PE floor)
    med = samples[len(samples) // 2]
    return y, med


def bench(inputs, iters=20, warmup=2):
    """Amortized wall-clock timing with device-resident inputs.

    Returns (y_full, per_iter_ns)."""
    import time
    import jax
    from jax.sharding import NamedSharding, PartitionSpec

    runner = _get_runner()
    in_maps = _host_prep(**inputs)
    concat_in = _concat_inputs(runner, in_maps)
    sharding = NamedSharding(runner["mesh"], PartitionSpec("core"))
    dev_in = [jax.device_put(a, sharding) for a in concat_in]
    zeros = [np.zeros((N_CORES * z.shape[0], *z.shape[1:]), z.dtype)
             for z in runner["zero_outs"]]
    dev_zeros = [jax.device_put(z, sharding) for z in zeros]

    outs = None
    for _ in range(warmup):
        outs = runner["sharded"](*dev_in, *dev_zeros)
        jax.block_until_ready(outs)
    t0 = time.perf_counter()
    keep = []
    for _ in range(iters):
        keep.append(runner["sharded"](*dev_in, *dev_zeros))
    jax.block_until_ready(keep[-1])
    t1 = time.perf_counter()
    per_iter_ns = (t1 - t0) / iters * 1e9

    (yaval,) = runner["out_avals"]
    y_all = np.asarray(outs[0]).reshape(N_CORES, *yaval.shape)
    y = y_all.reshape(TOK, D_DIM).reshape(B_DIM, S_DIM, D_DIM)
    return y, per_iter_ns
